# revision 30
# baseline (speedup 1.0000x reference)
"""AttentionPool TRN2 kernel.

Problem: B=2048, S=512, D=128, H=4, T=8 (Q = T*H = 32), C=64.
  k = keys @ Wk^T ; v = keys @ Wv^T
  q = q_flat + (ctx @ Wc^T + bc).reshape(B, Q, D)
  attn = (q @ k^T) * scale * inv_t[q] - slopes[q] * games_ago[s]
  out  = softmax_masked(attn) @ v            -> [B, T, H*D]

Structure (v2):
  - Host pre-casts keys to f16 and ships BOTH orientations:
      kt [rows, D, S]        (logits rhs: contract d)
      kp [rows, 128, 4, 128] (pass2 stationary tiles, s = c*128 + p)
    Same DMA bytes as one f32 copy, but no on-device cast and no PE
    key transposes.
  - All additive logit terms (ALiBi slope*s, -slope*(n-1) shift, mask)
    ride a single [6,*] matmul into the logits PSUM: rows 0-3 select
    MASK_NEG*mask[r,s], row 4 adds SC*slope_q*s, row 5 adds the
    per-(r,q) constant -(MASK_NEG + SC*slope_q*(n_r-1)) (host-computed
    from the mask). Softmax then needs no row-max: true logits <= ~2.
  - exp on scalar engine -> f16 weights + f32 row sums in one pass.
  - w^T via f16 PE transposes; pooled = (w @ keys) @ Wv^T in f16.

Sharding: pure data parallel over batch, 256 rows/core on 8 cores.
"""

import sys

if "/opt/trn_rl_repo" not in sys.path:
    sys.path.insert(0, "/opt/trn_rl_repo")

import numpy as np

import concourse.bacc as bacc
import concourse.bass as bass
import concourse.tile as tile
from concourse import mybir
from concourse.bass_utils import run_bass_kernel_spmd

B, S, D, H, T, C = 2048, 512, 128, 4, 8, 64
Q = T * H  # 32
N_CORES = 8
ROWS = B // N_CORES  # 256 rows per core
GRP = 4  # batch rows per group -> 4*32 = 128 partitions
BLK = 128  # rows per block (ctx/QKT staging)
SC = 64.0  # power-of-two prescale keeping f16 operands in normal range
MASK_NEG = 16384.0  # f16-exact; /SC = 256 pushes masked logits below -126

F32 = mybir.dt.float32
F16 = mybir.dt.float16

NCH = S // 128  # 4 s-chunks


def _emit(nc, tc, rows, cc):
    """Emit the per-core program for `rows` batch rows (rows % GRP == 0).

    kk[row] packs both key orientations: [:, 0, :] = keys^T row ([d, s]),
    [:, 1, :] = pass2 tiles ([p, (c d)], s = c*128 + p). m6 packs the
    bias/mask matmul rhs (cols 0..S-1) and lhsT (cols S..S+127).

    cc[g] in 1..4 is the number of 128-wide s-chunks group g touches.
    Rows are host-sorted by length so every skipped chunk is fully
    masked; its softmax terms are exact zeros, so the result is
    identical to the full computation.
    """
    kk_d = nc.declare_dram_parameter(
        "kk", [rows // GRP, 128, GRP, 2, S], F16, isOutput=False
    )
    ctx_d = nc.declare_dram_parameter("ctx", [rows, C], F32, isOutput=False)
    m6_d = nc.declare_dram_parameter("m6", [rows // GRP, 6, S + 128], F16, isOutput=False)
    maug_d = nc.declare_dram_parameter("maug", [C + 1, Q, D], F16, isOutput=False)
    wvt_d = nc.declare_dram_parameter("wvt", [D, D], F16, isOutput=False)
    id16_d = nc.declare_dram_parameter("id16", [128, 128], F16, isOutput=False)
    id32_d = nc.declare_dram_parameter("id32", [128, 128], F32, isOutput=False)
    out_d = nc.declare_dram_parameter("out", [rows, Q * D], F32, isOutput=True)

    kk_ap = kk_d.ap()
    ctx_ap = ctx_d.ap()
    out_ap = out_d.ap()

    n_blk = (rows + BLK - 1) // BLK

    import contextlib

    with contextlib.ExitStack() as ctx:
        singles = ctx.enter_context(tc.tile_pool(name="singles", bufs=1))
        kpool = ctx.enter_context(tc.tile_pool(name="kpool", bufs=6))
        mpool = ctx.enter_context(tc.tile_pool(name="mpool", bufs=6))
        blkpool = ctx.enter_context(tc.tile_pool(name="blkpool", bufs=2))
        qktpool = ctx.enter_context(tc.tile_pool(name="qktpool", bufs=2))
        work = ctx.enter_context(tc.tile_pool(name="work", bufs=4))
        small = ctx.enter_context(tc.tile_pool(name="small", bufs=4))
        ps = ctx.enter_context(tc.tile_pool(name="ps", bufs=1, space="PSUM"))

        # ---- constants (loaded once) ----
        maug_sb = singles.tile([C + 1, Q, D], F16)
        nc.sync.dma_start(out=maug_sb, in_=maug_d.ap())
        wvt_sb = singles.tile([D, D], F16)
        nc.sync.dma_start(out=wvt_sb, in_=wvt_d.ap())
        id16_sb = singles.tile([128, 128], F16)
        nc.sync.dma_start(out=id16_sb, in_=id16_d.ap())
        id32_sb = singles.tile([128, 128], F32)
        nc.sync.dma_start(out=id32_sb, in_=id32_d.ap())

        # ---- prologue: conditioned queries qk'^T for every block ----
        qkt_blocks = []
        for blk in range(n_blk):
            r0 = blk * BLK
            bn = min(BLK, rows - r0)
            assert bn % GRP == 0

            ctx_sb = blkpool.tile([BLK, C], F32, tag="ctx")
            nc.sync.dma_start(out=ctx_sb[:bn], in_=ctx_ap[r0 : r0 + bn])
            ctxt_ps = ps.tile([C, BLK], F32, tag="smallf32", bufs=1)
            nc.tensor.transpose(ctxt_ps[:, :bn], ctx_sb[:bn], id32_sb[:bn, :bn])
            ctxt_sb = blkpool.tile([C + 1, BLK], F16, tag="ctxt")
            nc.vector.tensor_copy(out=ctxt_sb[:C, :bn], in_=ctxt_ps[:, :bn])
            nc.vector.memset(ctxt_sb[C : C + 1, :bn], 1.0)

            # qk'^T for the block: [D, bn, Q] f16 (prescaled by SC*scale*inv_t)
            qkt_sb = qktpool.tile([D, BLK, Q], F16, tag="qkt")
            for q in range(Q):
                qkt_ps = ps.tile([D, BLK], F32, tag="smallf32", bufs=1)
                nc.tensor.matmul(
                    qkt_ps[:, :bn], maug_sb[:, q, :], ctxt_sb[:, :bn],
                    start=True, stop=True,
                )
                nc.vector.tensor_copy(out=qkt_sb[:, :bn, q], in_=qkt_ps[:, :bn])
            qkt_blocks.append(qkt_sb)

        n_grp_total = rows // GRP
        PF = 2  # software prefetch distance (groups)
        staged = {}

        def _load_group(g):
            if g >= n_grp_total or g in staged:
                return
            sl = cc[g] * 128
            kkg = kpool.tile([128, GRP, 2, S], F16, tag="kk", name=f"kk_{g}")
            nc.sync.dma_start(
                out=kkg[:, :, :, :sl],
                in_=kk_ap[g, :, :, :, :sl],
            )
            m6 = mpool.tile([6, S + 128], F16, tag="m6", name=f"m6_{g}")
            nc.gpsimd.dma_start(out=m6, in_=m6_d.ap()[g])
            staged[g] = (kkg, m6)

        for g in range(PF):
            _load_group(g)

        # Three-stage software pipeline: iteration g runs logits/mask/exp
        # for group g while the PE finishes wT/pass2/pooled for group
        # g-2. The PE never waits on a recent EXP.
        DEPTH = 1
        pending = []
        for g in range(n_grp_total + DEPTH):
            if g < n_grp_total:
                g0 = g * GRP  # absolute row of this group
                qkt_sb = qkt_blocks[g0 // BLK]
                gl = g0 % BLK  # row offset inside the block
                _load_group(g + PF)
                kkg, m6 = staged.pop(g)
                ncg = cc[g]
                sl = ncg * 128

                # ---- logits psum: qk' . k^T  (+ bias/mask matmul) ----
                lg_ps = ps.tile([128, S], F32, tag="logits", bufs=2)
                for r in range(GRP):
                    nc.tensor.matmul(
                        lg_ps[32 * r : 32 * (r + 1), :sl],
                        qkt_sb[:, gl + r, :],
                        kkg[:, r, 0, :sl],
                        start=True, stop=False,
                        tile_position=(0, 32 * r),
                        skip_group_check=True,
                    )
                nc.tensor.matmul(
                    lg_ps[:, :sl], m6[:, S : S + 128], m6[:, :sl],
                    start=False, stop=True,
                    skip_group_check=True,
                )

                # ---- softmax (no row max needed: true logits <= ~2) ----
                e16 = work.tile([128, S], F16, tag="e16")
                sum_sb = small.tile([128, 1], F32, tag="sum")
                nc.scalar.activation(
                    out=e16[:, :sl], in_=lg_ps[:, :sl],
                    func=mybir.ActivationFunctionType.Exp,
                    scale=1.0 / SC, accum_out=sum_sb,
                )
                pending.append((g0, kkg, e16, sum_sb, ncg))

            if len(pending) > DEPTH or (g >= n_grp_total and pending):
                pg0, pkkg, pe16, psum_sb, pncg = pending.pop(0)

                # ---- w^T: [s_in_chunk, c, rq] f16 via PE transposes;
                #      per-chunk psum->sbuf copies so pass2 can start
                #      after the first chunk ----
                wt_ps = ps.tile([128, NCH, 128], F16, tag="wtps", bufs=2)
                wt16 = work.tile([128, NCH, 128], F16, tag="wt")
                for c in range(pncg):
                    nc.tensor.transpose(
                        wt_ps[:, c, :], pe16[:, c * 128 : (c + 1) * 128], id16_sb
                    )
                    nc.vector.tensor_copy(out=wt16[:, c], in_=wt_ps[:, c, :])

                # ---- pass 2: pk[d, rq] = sum_s keys[s,d] * w[s,rq] ----
                pk_ps = ps.tile([128, 128], F32, tag="pk", bufs=2)
                for r in range(GRP):
                    for c in range(pncg):
                        nc.tensor.matmul(
                            pk_ps[:, 32 * r : 32 * (r + 1)],
                            pkkg[:, r, 1, c * 128 : (c + 1) * 128],
                            wt16[:, c, 32 * r : 32 * (r + 1)],
                            start=(c == 0), stop=(c == pncg - 1),
                            skip_group_check=True,
                        )
                pkt16 = work.tile([128, 128], F16, tag="pkt")
                nc.vector.tensor_copy(out=pkt16, in_=pk_ps)

                # ---- pooled[rq, e] = pk^T @ Wv^T, scaled by 1/rowsum ----
                po_ps = ps.tile([128, 128], F32, tag="po", bufs=1)
                nc.tensor.matmul(po_ps, pkt16, wvt_sb, start=True, stop=True)

                rs_sb = small.tile([128, 1], F32, tag="rs")
                nc.vector.reciprocal(rs_sb, psum_sb)
                o_sb = work.tile([128, 128], F32, tag="o")
                nc.vector.tensor_scalar(
                    out=o_sb, in0=po_ps, scalar1=rs_sb, scalar2=None,
                    op0=mybir.AluOpType.mult,
                )
                nc.gpsimd.dma_start(
                    out=out_ap[pg0 : pg0 + GRP].rearrange("r (q e) -> (r q) e", e=D),
                    in_=o_sb,
                )


def _build(rows, cc):
    nc = bacc.Bacc(trn_type="TRN2", target_bir_lowering=False, debug=False)
    with tile.TileContext(nc) as tc:
        _emit(nc, tc, rows, cc)
    nc.compile()
    return nc


def _schedule(mask_b):
    """Sort rows by length (descending), deal into slots of N_CORES*GRP.

    Returns (perms, cc): perms[c] is core c's row order (global indices),
    cc[k] = chunks needed by slot k — identical across cores.
    """
    n_real = mask_b.sum(axis=1)
    order = np.argsort(-n_real, kind="stable")
    n_slot = B // (N_CORES * GRP)
    cc = []
    for k in range(n_slot):
        nmax = int(n_real[order[k * N_CORES * GRP]])
        cc.append(max(1, -(-nmax // 128)))
    perms = []
    for c in range(N_CORES):
        idx = np.concatenate(
            [
                order[k * N_CORES * GRP + c * GRP : k * N_CORES * GRP + (c + 1) * GRP]
                for k in range(n_slot)
            ]
        )
        perms.append(idx)
    return perms, tuple(cc)


def host_consts(queries, Wk, log_temperature, Wc, bc, Wv):
    """Fold projections/scales into small host-side constants."""
    queries = np.asarray(queries, np.float64)
    Wk = np.asarray(Wk, np.float64)
    Wc = np.asarray(Wc, np.float64)
    bc = np.asarray(bc, np.float64)
    Wv = np.asarray(Wv, np.float64)
    lt = np.asarray(log_temperature, np.float64)

    scale = D ** -0.5
    inv_t = np.repeat(np.exp(-lt), H)  # [Q]
    s_q = scale * inv_t  # [Q]

    q_eff = queries.reshape(Q, D) + bc.reshape(Q, D)  # [Q, D]
    qk0 = q_eff @ Wk  # [Q, D]
    # maug[c, q, d]: rows 0..C-1 = SC*s_q * (Wc_q^T @ Wk); row C = SC*s_q * qk0
    maug = np.empty((C + 1, Q, D), np.float64)
    for q in range(Q):
        Wc_q = Wc[q * D : (q + 1) * D, :]  # [D(e), C]
        maug[:C, q, :] = (Wc_q.T @ Wk) * (SC * s_q[q])
        maug[C, q, :] = qk0[q] * (SC * s_q[q])

    return dict(
        maug=maug.astype(np.float16),
        wvt=np.ascontiguousarray(Wv.T).astype(np.float16),
        id16=np.eye(128, dtype=np.float16),
        id32=np.eye(128, dtype=np.float32),
    )


def _slopes_q():
    slopes_h = 2.0 ** (-8.0 * (np.arange(H) + 1) / H)
    return np.tile(slopes_h, T)  # [Q]


def make_in_maps(keys, mask, context, consts, rows, n_cores, perms):
    keys16 = np.asarray(keys, np.float32).astype(np.float16)  # [B, S, D]
    # kk[:, :, 0, :] = keys^T ([d, s]); kk[:, :, 1, :] = pass2 tiles
    # ([p, (c d)], s = c*128 + p). One DMA per group covers both.
    kk = np.empty((B, 128, 2, S), np.float16)
    kk[:, :, 0, :] = keys16.transpose(0, 2, 1)
    kk[:, :, 1, :] = (
        keys16.reshape(B, NCH, 128, D).transpose(0, 2, 1, 3).reshape(B, 128, S)
    )
    mask_b = np.asarray(mask).astype(bool)
    ctx = np.asarray(context, np.float32)

    n_real = mask_b.sum(axis=1).astype(np.float64)  # [B]
    slopes = _slopes_q()  # [Q]
    n_grp = rows // GRP

    # m6[:, :, :S] (rhs)  rows 0-3: mask[r] f16; row 4: s values; row 5: ones
    # m6[:, :, S:] (lhsT) rows 0-3: MASK_NEG on the r-th 32-col block;
    #      row 4: SC*slope_q ; row 5: -(MASK_NEG + SC*slope_q*(n_r - 1))
    svals = np.arange(S, dtype=np.float16)
    slope_row = np.tile(SC * slopes, 128 // Q).astype(np.float16)

    in_maps = []
    for i in range(n_cores):
        perm = perms[i]
        mk = mask_b[perm]  # [rows, S]
        nr = n_real[perm]  # [rows]
        m6 = np.zeros((n_grp, 6, S + 128), np.float16)
        m6[:, :GRP, :S] = mk.astype(np.float16).reshape(n_grp, GRP, S)
        m6[:, 4, :S] = svals
        m6[:, 5, :S] = 1.0
        m6[:, 4, S:] = slope_row
        for r in range(GRP):
            m6[:, r, S + 32 * r : S + 32 * (r + 1)] = MASK_NEG
            m6[:, 5, S + 32 * r : S + 32 * (r + 1)] = -(
                MASK_NEG + SC * slopes[None, :] * (nr[r::GRP, None] - 1.0)
            ).astype(np.float16)
        kk_core = np.ascontiguousarray(
            kk[perm].reshape(n_grp, GRP, 128, 2, S).transpose(0, 2, 1, 3, 4)
        )
        in_maps.append(
            dict(
                kk=kk_core,
                ctx=np.ascontiguousarray(ctx[perm]),
                m6=m6,
                **consts,
            )
        )
    return in_maps


_cache = {}


def run(keys, mask, context, queries, Wk, Wv, log_temperature, Wc, bc,
        trace=False, **kw):
    consts = host_consts(queries, Wk, log_temperature, Wc, bc, Wv)
    mask_b = np.asarray(mask).astype(bool)
    perms, cc = _schedule(mask_b)
    key = (ROWS, cc)
    if key not in _cache:
        _cache[key] = _build(ROWS, cc)
    nc = _cache[key]
    in_maps = make_in_maps(keys, mask, context, consts, ROWS, N_CORES, perms)
    res = run_bass_kernel_spmd(nc, in_maps, core_ids=list(range(N_CORES)),
                               trace=trace, **kw)
    out = np.empty((B, Q * D), np.float32)
    for i in range(N_CORES):
        out[perms[i]] = res.results[i]["out"]
    return out.reshape(B, T, H * D), res


def kernel(keys, mask, context, queries, Wk, Wv, log_temperature, Wc, bc):
    out, _ = run(keys, mask, context, queries, Wk, Wv, log_temperature, Wc, bc)
    return out
